# revision 18
# baseline (speedup 1.0000x reference)
"""Trainium2 Bass kernel for a 3-layer GAT block (DeepGATBlockV2).

Strategy (8-core SPMD, nodes partitioned by dst range):
  - ALL per-core constants + inputs are packed into ONE int16 blob input.
  - Per layer, each core builds packed table rows for its 2500-node shard:
    256 B/row = [x as fp8e4 (128 B) | a_s bf16 (8 B) | a_d bf16 (8 B) |
    pad (112 B)].  AllGather -> full [N, 256 B] table in DRAM (5.12 MB).
  - Edges (incl. self loops) are sorted by dst on the host, bucketed into
    per-core dst-blocks of 125 nodes, padded to a uniform C chunks of 128
    edge slots.  Per block ONE merged dma_gather fetches 2*slots rows
    (src rows then dst rows, 256 B each).
  - q = a_s[src]+a_d[dst] (DVE 2x bf16); w = exp(leaky_relu(q)) on ACT,
    written twice (wex2 pairs) so the one-hot builds hit the DVE 2x mode:
    s0[e, (n/2, 2)] and sh[e, (h, n/2, 2)] use pair-padded width NB=126
    with all-packed innermost dims.  PE accumulates
    s_hT[f, (h,nb)] += Xg(fp8).T @ sh(bf16) and den[(h,nb)] += wex @ s0.
  - Block post: y = s_hT * bcast(1/den) (0.25 head-mean folded into
    W_gat), attT = sum_h W_h.T @ y_h + bias; residual + RMSNorm + FFN
    (bf16 matmuls) + RMSNorm, all feature-major.  Phase A of the NEXT
    layer is emitted right after each block's post so table rows stream
    out while later blocks still aggregate.
  - RMSNorm rsqrt = exp(-0.5*ln(ms+eps)); one activation table set.
"""

import functools

import numpy as np

import concourse.bass as bass
import concourse.bacc as bacc
import concourse.hw_specs as hw_specs
import concourse.tile as tile
from concourse import mybir
from concourse.bass_utils import run_bass_kernel_spmd

F32 = mybir.dt.float32
BF16 = mybir.dt.bfloat16
I16 = mybir.dt.int16
I8 = mybir.dt.int8
F8 = mybir.dt.float8e4
AOT = mybir.AluOpType
ACT = mybir.ActivationFunctionType

EPS = 1.1920929e-07
NEG_SLOPE = 0.2
ABLATE = set()  # timing-ablation flags

# ---- activation-table forcing ----------------------------------------
_COMBINED_SET = "natural_log_exp_and_others"
_orig_get_tables = hw_specs.get_activation_tables


@functools.cache
def _forced_tables(arch):
    t = {k: set(v) for k, v in _orig_get_tables(arch).items()}
    used = {ACT.Exp, ACT.Ln, ACT.Prelu, ACT.Relu, ACT.Copy, ACT.Identity,
            ACT.MemsetZero}
    if _COMBINED_SET in t and used <= t[_COMBINED_SET]:
        for name, funcs in t.items():
            if name != _COMBINED_SET:
                funcs -= used
    return t


hw_specs.get_activation_tables = _forced_tables
bacc.get_activation_tables = _forced_tables


def _np_bf16():
    import ml_dtypes
    return ml_dtypes.bfloat16


class _Blob:
    """Packs 2-D arrays into one flat int16 buffer, 256B-aligned rows."""

    def __init__(self):
        self.parts = []
        self.off = 0  # int16 elems
        self.secs = {}

    def add(self, name, arr):
        a = np.ascontiguousarray(arr)
        b = a.view(np.int16)
        rows, cols = b.shape
        pad = (-self.off) % 128
        if pad:
            self.parts.append(np.zeros(pad, np.int16))
            self.off += pad
        self.secs[name] = (self.off, rows, cols)
        self.parts.append(b.reshape(-1))
        self.off += rows * cols

    def finish(self):
        pad = (-self.off) % 128
        if pad:
            self.parts.append(np.zeros(pad, np.int16))
            self.off += pad
        return np.concatenate(self.parts)


def host_prep(inputs, cfg):
    """Returns (in_maps, C) -- per-core single-blob inputs."""
    N, E, CORES = cfg["N"], cfg["E"], cfg["CORES"]
    SHARD, BLK, BLOCKS = cfg["SHARD"], cfg["BLK"], cfg["BLOCKS"]
    L, D, H = cfg["L"], cfg["D"], cfg["H"]
    NB = BLK + 1  # pair-padded one-hot width (126)
    bf16 = _np_bf16()

    x = np.ascontiguousarray(np.asarray(inputs["x"], np.float32))
    ei = np.asarray(inputs["edge_index"], np.int64)
    src = ei[0]
    dst = ei[1]
    loops = np.arange(N, dtype=np.int64)
    src = np.concatenate([src, loops])
    dst = np.concatenate([dst, loops])
    order = np.argsort(dst, kind="stable")
    src, dst = src[order], dst[order]

    nblk_total = N // BLK
    blk_of = dst // BLK
    counts = np.bincount(blk_of, minlength=nblk_total)
    C = int(np.ceil(counts.max() / 128))
    cfg["C"] = C
    slots = C * 128

    srcs = np.zeros((CORES, BLOCKS, slots), np.int64)
    dloc = np.full((CORES, BLOCKS, slots), -1.0, np.float32)
    dsts = np.zeros((CORES, BLOCKS, slots), np.int64)
    starts = np.concatenate([[0], np.cumsum(counts)])
    for b in range(nblk_total):
        core, blk = b // BLOCKS, b % BLOCKS
        s, e = int(starts[b]), int(starts[b + 1])
        n = e - s
        srcs[core, blk, :n] = src[s:e]
        dsts[core, blk, :n] = dst[s:e]
        dsts[core, blk, n:] = b * BLK  # valid row for pad reads
        dloc[core, blk, :n] = (dst[s:e] - b * BLK).astype(np.float32)

    # merged per-block index list: [srcs | dsts], wrapped for dma_gather
    both = np.concatenate([srcs, dsts], axis=2)  # [CORES, BLOCKS, 2*slots]

    def wrap_idx(a):
        # a: [BLOCKS, ns] int -> int16 [128, BLOCKS * ns//16]
        ns = a.shape[1]
        a16 = a.reshape(BLOCKS, ns // 16, 16).transpose(0, 2, 1)
        a16 = a16.reshape(1, BLOCKS * 16, ns // 16)
        cols = np.concatenate(
            [a16[0, b * 16:(b + 1) * 16, :] for b in range(BLOCKS)],
            axis=1)  # [16, BLOCKS*ns//16]
        assert a.max() < 2 ** 15
        return np.tile(cols.astype(np.int16), (8, 1))

    # dloc layout [128, BLOCKS*C]: [p, b*C + ch] = slot ch*128+p of block b
    dloc_t = dloc.reshape(CORES, BLOCKS, C, 128).transpose(0, 3, 1, 2) \
                 .reshape(CORES, 128, BLOCKS * C)
    dloc2 = np.repeat(dloc_t, 2, axis=2)  # [CORES, 128, BLOCKS*C*2]

    Wg = np.asarray(inputs["W_gat"], np.float32)     # [L, D, H*D]
    a_s = np.asarray(inputs["att_src"], np.float32)  # [L, H, D]
    a_d = np.asarray(inputs["att_dst"], np.float32)
    wasd = np.zeros((D, L * 2 * H), np.float32)
    for l in range(L):
        for h in range(H):
            Wh = Wg[l][:, h * D:(h + 1) * D]
            wasd[:, l * 2 * H + h] = Wh @ a_s[l, h]
            wasd[:, l * 2 * H + H + h] = Wh @ a_d[l, h]

    def col3(name):  # [L, D] -> [D, L]
        return np.ascontiguousarray(np.asarray(inputs[name], np.float32).T)

    blob = _Blob()
    # 0.25 head-mean folded into W_gat; [D, L*H*D] d-major
    blob.add("wgat", (0.25 * Wg).transpose(1, 0, 2).reshape(D, L * H * D)
             .astype(bf16))
    blob.add("w1", np.asarray(inputs["W1"], np.float32)
             .transpose(1, 0, 2).reshape(D, L * D).astype(bf16))
    blob.add("w2", np.asarray(inputs["W2"], np.float32)
             .transpose(1, 0, 2).reshape(D, L * D).astype(bf16))
    blob.add("wasd", wasd)
    blob.add("bg", col3("bias_gat"))
    blob.add("b1", col3("b1"))
    blob.add("b2", col3("b2"))
    blob.add("n1", np.asarray(inputs["norm1_w"], np.float32).reshape(1, -1))
    blob.add("n2", np.asarray(inputs["norm2_w"], np.float32).reshape(1, -1))
    blob.add("iota", np.tile(np.arange(NB, dtype=np.float32), (128, 1))
             .astype(bf16))
    blob.add("ident", np.eye(128, dtype=np.float32))
    blob.add("onesf", np.ones((128, 2), np.float32))  # col 0 used
    # head-selector for denominator broadcast: hsel[k, h*128+m] = (k==h)
    blob.add("hsel", np.eye(H, dtype=np.float32).repeat(128, axis=1)
             .astype(bf16))
    common_len = blob.off
    common_parts = list(blob.parts)
    common_secs = dict(blob.secs)

    in_maps = []
    blob_len = None
    for c in range(CORES):
        bl = _Blob()
        bl.parts = list(common_parts)
        bl.off = common_len
        bl.secs = dict(common_secs)
        bl.add("idx", wrap_idx(both[c]))
        bl.add("dloc2", dloc2[c].astype(bf16))
        bl.add("xin", x[c * SHARD:(c + 1) * SHARD])
        buf = bl.finish()
        blob_len = len(buf)
        cfg["SECS"] = bl.secs
        in_maps.append({"blob": buf})
    cfg["BLOB_LEN"] = blob_len
    return in_maps, C


def build_program(cfg, debug=False):
    N, CORES = cfg["N"], cfg["CORES"]
    SHARD, BLK, BLOCKS, C = cfg["SHARD"], cfg["BLK"], cfg["BLOCKS"], cfg["C"]
    L, D, H = cfg["L"], cfg["D"], cfg["H"]
    SECS = cfg["SECS"]
    NB = BLK + 1      # pair-padded block width (126)
    TB = 256          # table row bytes
    slots = C * 128
    NQ = cfg.get("NQ", 4)
    RP = cfg.get("REPS", 1)

    nc = bacc.Bacc("TRN2", target_bir_lowering=False, debug=debug,
                   num_devices=CORES, num_swdge_queues=NQ,
                   dynamic_dma_scratch_size=40960)

    blob = nc.dram_tensor("blob", [cfg["BLOB_LEN"]], I16,
                          kind="ExternalInput").ap()
    out = nc.dram_tensor("out", [SHARD, D], F32, kind="ExternalOutput").ap()

    def sec(name, dt=F32):
        off, rows, cols = SECS[name]
        v = blob[off:off + rows * cols].rearrange("(r c) -> r c", c=cols)
        return v if dt == I16 else v.bitcast(dt)

    with tile.TileContext(nc) as tc:
        with tc.tile_pool(name="persist", bufs=1) as pp, \
             tc.tile_pool(name="dram", bufs=1, space="DRAM") as dp, \
             tc.tile_pool(name="gath", bufs=5) as gp, \
             tc.tile_pool(name="sc", bufs=4) as scp, \
             tc.tile_pool(name="chunk", bufs=3) as cp, \
             tc.tile_pool(name="post", bufs=2) as pop, \
             tc.tile_pool(name="psA", bufs=1, space="PSUM") as psA, \
             tc.tile_pool(name="psB", bufs=2, space="PSUM") as psB:

            # ---- persistent SBUF ----
            idx_s = pp.tile([128, BLOCKS * 2 * slots // 16], I16)
            dloc2_s = pp.tile([128, BLOCKS * C * 2], BF16)
            wasd_s = pp.tile([128, L * 2 * H], F32)
            wgat_s = pp.tile([128, L * H * D], BF16)
            w1_s = pp.tile([128, L * D], BF16)
            w2_s = pp.tile([128, L * D], BF16)
            bg_s = pp.tile([128, L], F32)
            b1_s = pp.tile([128, L], F32)
            b2_s = pp.tile([128, L], F32)
            n1_s = pp.tile([1, L * D], F32)
            n2_s = pp.tile([1, L * D], F32)
            iota_s = pp.tile([128, NB], BF16)
            ident_s = pp.tile([128, 128], F32)
            onesf_s = pp.tile([128, 2], F32)
            hsel_s = pp.tile([4, 4 * 128], BF16)
            xT = pp.tile([128, SHARD], F32)
            eps_s = pp.tile([1, 1], F32)

            dma = nc.sync.dma_start
            dma(idx_s[:], sec("idx", I16))
            dma(dloc2_s[:], sec("dloc2", BF16))
            dma(wasd_s[:], sec("wasd"))
            dma(wgat_s[:], sec("wgat", BF16))
            dma(w1_s[:], sec("w1", BF16))
            dma(w2_s[:], sec("w2", BF16))
            dma(bg_s[:], sec("bg"))
            dma(b1_s[:], sec("b1"))
            dma(b2_s[:], sec("b2"))
            dma(n1_s[:], sec("n1"))
            dma(n2_s[:], sec("n2"))
            dma(iota_s[:], sec("iota", BF16))
            dma(ident_s[:], sec("ident"))
            dma(onesf_s[:], sec("onesf"))
            dma(hsel_s[:], sec("hsel", BF16))
            nc.vector.memset(eps_s[:], EPS)
            xin = sec("xin", F32)

            # ---- DRAM tables for gather + collective ----
            tshard = dp.tile([SHARD, TB], I8)
            if CORES > 1:
                tfulls = [dp.tile([N, TB], I8, addr_space="Shared",
                                  tag=f"tfull{i}", name=f"tfull{i}")
                          for i in range(L * RP)]
            else:
                tfulls = [tshard] * (L * RP)

            # ---- init: transpose input shard to feature-major xT ----
            for b in range(BLOCKS):
                xr = gp.tile([BLK, D], F32, tag="xr")
                nc.sync.dma_start(xr[:], xin[b * BLK:(b + 1) * BLK, :])
                ps_t = psB.tile([D, BLK], F32, tag="pb")
                nc.tensor.transpose(ps_t[:], xr[:], ident_s[:BLK, :BLK])
                nc.scalar.copy(xT[:, b * BLK:(b + 1) * BLK], ps_t[:])

            def rmsnorm(z, nw_row, tag, zout=None):
                """z: SBUF [D, BLK] f32 -> z * rsqrt(mean(z^2)+eps) * w."""
                zsq = pop.tile([D, BLK], F32, tag=f"zsq{tag}")
                nc.vector.tensor_mul(zsq[:], z[:], z[:])
                ps_ss = psB.tile([1, BLK], F32, tag="pb_ss", bufs=1)
                nc.tensor.matmul(ps_ss[:], onesf_s[:, 0:1], zsq[:],
                                 start=True, stop=True)
                lnm = pop.tile([1, BLK], F32, tag=f"lnm{tag}")
                nc.scalar.activation(lnm[:], ps_ss[:], ACT.Ln,
                                     scale=1.0 / D, bias=eps_s[:])
                rin = pop.tile([1, BLK], F32, tag=f"rin{tag}")
                nc.scalar.activation(rin[:], lnm[:], ACT.Exp, scale=-0.5)
                ps_rb = psB.tile([D, BLK], F32, tag="pb")
                nc.tensor.matmul(ps_rb[:], nw_row, rin[:],
                                 start=True, stop=True)
                zn = zout if zout is not None else pop.tile(
                    [D, BLK], F32, tag=f"zn{tag}")
                nc.vector.tensor_mul(zn if zout is not None else zn[:],
                                     z[:], ps_rb[:])
                return zn

            def phase_a(l, b):
                """Write packed table rows for block b, layer l."""
                xb = xT[:, b * BLK:(b + 1) * BLK]
                ps_a = psB.tile([BLK, 2 * H], F32, tag="pb")
                nc.tensor.matmul(ps_a[:], xb,
                                 wasd_s[:, l * 2 * H:(l + 1) * 2 * H],
                                 start=True, stop=True)
                ps_x = psB.tile([BLK, D], F32, tag="pb")
                nc.tensor.transpose(ps_x[:], xb, ident_s[:])
                tt = gp.tile([BLK, TB], I8, tag="tt")
                nc.scalar.copy(tt[:, 0:128].bitcast(F8), ps_x[:])
                nc.scalar.copy(tt[:, 128:144].bitcast(BF16), ps_a[:])
                nc.vector.memset(tt[:, 144:TB], 0)
                nc.sync.dma_start(tshard[b * BLK:(b + 1) * BLK, :], tt[:])

            # ---- phase C1: issue the gathers for one block ----
            def gather_phase(i, b):
                tf = tfulls[i]
                gab = gp.tile([128, 2 * C * TB], I8, tag="gab")
                gv = gab[:].rearrange("p (c e) -> p c e", e=TB)
                ic0 = b * (2 * slots // 16)
                icm = ic0 + slots // 16
                ic1 = ic0 + 2 * (slots // 16)
                nc.gpsimd.dma_gather(
                    gv[:, 0:C, :], tf[:], idx_s[:, ic0:icm],
                    num_idxs=slots, num_idxs_reg=slots,
                    elem_size=TB, queue_num=(2 * b) % NQ,
                    single_packet=False)
                nc.gpsimd.dma_gather(
                    gv[:, C:2 * C, :], tf[:], idx_s[:, icm:ic1],
                    num_idxs=slots, num_idxs_reg=slots,
                    elem_size=TB, queue_num=(2 * b + 1) % NQ,
                    single_packet=False)
                return gv

            # ---- phase C2a: per-edge attention scores (DVE+ACT only) ----
            def score_phase(b, gv):
                q = scp.tile([128, C * H], BF16, tag="q")
                lr = scp.tile([128, C * H], BF16, tag="lr")
                wex2 = scp.tile([128, C * H * 2], BF16, tag="wex2")
                nc.vector.tensor_add(
                    q[:].rearrange("p (c h) -> p c h", h=H),
                    gv[:, 0:C, 128:136].bitcast(BF16),
                    gv[:, C:2 * C, 136:144].bitcast(BF16))
                nc.scalar.activation(lr[:], q[:], ACT.Prelu,
                                     alpha=NEG_SLOPE)
                w2v = wex2[:].rearrange("p (c h two) -> p c h two", h=H,
                                        two=2)
                nc.scalar.activation(
                    w2v[:, :, :, 0].rearrange("p c h -> p (c h)"),
                    lr[:], ACT.Exp)
                nc.scalar.activation(
                    w2v[:, :, :, 1].rearrange("p c h -> p (c h)"),
                    lr[:], ACT.Exp)
                return w2v

            # ---- phase C2b: one-hots + PE aggregation ----
            def math_phase(i, b, gv, w2v):
                s0 = cp.tile([128, C * NB], BF16, tag="s0")
                sh = cp.tile([128, C * H * NB], BF16, tag="sh")
                nc.vector.tensor_tensor(
                    s0[:].rearrange("p (c n2 two) -> p c n2 two", two=2,
                                    n2=NB // 2),
                    dloc2_s[:, b * C * 2:(b + 1) * C * 2]
                        .rearrange("p (c two) -> p c two", two=2)
                        .unsqueeze(2).broadcast_to([128, C, NB // 2, 2]),
                    iota_s[:].rearrange("p (n2 two) -> p n2 two", two=2)
                        .unsqueeze(1).broadcast_to([128, C, NB // 2, 2]),
                    op=AOT.is_equal)
                nc.vector.tensor_tensor(
                    sh[:].rearrange("p (c h n2 two) -> p c h n2 two",
                                    h=H, two=2, n2=NB // 2),
                    s0[:].rearrange("p (c n2 two) -> p c n2 two", two=2,
                                    n2=NB // 2)
                        .unsqueeze(2).broadcast_to([128, C, H, NB // 2, 2]),
                    w2v.unsqueeze(3).broadcast_to([128, C, H, NB // 2, 2]),
                    op=AOT.mult)

                ps_all = psA.tile([D, H * NB], F32, tag="ps_all",
                                  name=f"ps_all_{i}_{b}", bufs=3)
                ps_den = psA.tile([H, NB], F32, tag="ps_den",
                                  name=f"ps_den_{i}_{b}", bufs=2)
                wex0 = w2v[:, :, :, 0]
                # den first: it only needs s0+wex, so PE can run it while
                # the big sh build is still in flight on DVE
                for ch in range(C):
                    nc.tensor.matmul(
                        ps_den[:], wex0[:, ch, :],
                        s0[:, ch * NB:(ch + 1) * NB],
                        start=(ch == 0), stop=(ch == C - 1))
                for ch in range(C):
                    nc.tensor.matmul(
                        ps_all[:], gv[:, ch, 0:128].bitcast(F8),
                        sh[:, ch * H * NB:(ch + 1) * H * NB],
                        start=(ch == 0), stop=(ch == C - 1))
                # 1/den broadcast to 128 partitions (runs during agg on PE)
                rden = pop.tile([H, BLK], BF16, tag="rden")
                with nc.allow_low_precision(reason="1/den fine in bf16"):
                    nc.vector.reciprocal(rden[:], ps_den[:, 0:BLK])
                ps_rb = psB.tile([128, H * BLK], F32, tag="pb")
                for h in range(H):
                    nc.tensor.matmul(ps_rb[:, h * BLK:(h + 1) * BLK],
                                     hsel_s[:, h * 128:(h + 1) * 128],
                                     rden[:], start=True, stop=True)
                rb = pop.tile([128, H * BLK], F32, tag="rb", bufs=4)
                nc.scalar.copy(rb[:], ps_rb[:])
                return ps_all, rb

            # ---- phase D: softmax-normalize + FFN for one block ----
            def post_phase(l, b, ps_all, rb):
                xb = xT[:, b * BLK:(b + 1) * BLK]
                yh = pop.tile([128, H * BLK], BF16, tag="yh")
                nc.vector.tensor_mul(
                    yh[:].rearrange("p (h n) -> p h n", n=BLK),
                    ps_all[:].rearrange("p (h n) -> p h n", n=NB)
                        [:, :, 0:BLK],
                    rb[:].rearrange("p (h n) -> p h n", n=BLK))
                ps_att = psB.tile([D, BLK], F32, tag="pb")
                for h in range(H):
                    nc.tensor.matmul(
                        ps_att[:],
                        wgat_s[:, (l * H + h) * D:(l * H + h + 1) * D],
                        yh[:, h * BLK:(h + 1) * BLK],
                        start=(h == 0), stop=(h == H - 1))

                z = pop.tile([D, BLK], F32, tag="z")
                nc.vector.scalar_tensor_tensor(
                    z[:], ps_att[:], bg_s[:, l:l + 1], xb,
                    op0=AOT.add, op1=AOT.add)
                zn1 = rmsnorm(z, n1_s[0:1, l * D:(l + 1) * D], "a")

                zn1h = pop.tile([D, BLK], BF16, tag="zn1h")
                nc.scalar.copy(zn1h[:], zn1[:])
                ps_f1 = psB.tile([D, BLK], F32, tag="pb")
                nc.tensor.matmul(ps_f1[:], w1_s[:, l * D:(l + 1) * D],
                                 zn1h[:], start=True, stop=True)
                f1 = pop.tile([D, BLK], BF16, tag="f1")
                nc.scalar.activation(f1[:], ps_f1[:], ACT.Relu,
                                     bias=b1_s[:, l:l + 1])
                ps_f2 = psB.tile([D, BLK], F32, tag="pb")
                nc.tensor.matmul(ps_f2[:], w2_s[:, l * D:(l + 1) * D],
                                 f1[:], start=True, stop=True)
                z3 = pop.tile([D, BLK], F32, tag="z3")
                nc.vector.scalar_tensor_tensor(
                    z3[:], ps_f2[:], b2_s[:, l:l + 1], zn1[:],
                    op0=AOT.add, op1=AOT.add)
                rmsnorm(z3, n2_s[0:1, l * D:(l + 1) * D], "b", zout=xb)

            # ---- main sequence ----
            for b in range(BLOCKS):
                phase_a(0, b)
            NLAY = L * RP
            AHEAD = 2  # gathers issued this many blocks early
            for i in range(NLAY):
                l = i % L
                if CORES > 1:
                    nc.gpsimd.collective_compute(
                        "AllGather", AOT.bypass,
                        replica_groups=[list(range(CORES))],
                        ins=[tshard.opt()], outs=[tfulls[i].opt()])
                gvs = {b: gather_phase(i, b) for b in range(AHEAD)}
                pend = []  # lag-2 post queue
                for b in range(BLOCKS):
                    if b + AHEAD < BLOCKS:
                        gvs[b + AHEAD] = gather_phase(i, b + AHEAD)
                    gv = gvs.pop(b)
                    w2v = score_phase(b, gv)
                    if len(pend) >= 2:
                        pb_, psall_, rb_ = pend.pop(0)
                        post_phase(l, pb_, psall_, rb_)
                        if i + 1 < NLAY:
                            phase_a((i + 1) % L, pb_)
                    pend.append((b, *math_phase(i, b, gv, w2v)))
                for pb_, psall_, rb_ in pend:
                    post_phase(l, pb_, psall_, rb_)
                    if i + 1 < NLAY:
                        phase_a((i + 1) % L, pb_)

            # ---- output: transpose back to node-major ----
            for b in range(BLOCKS):
                ps_o = psB.tile([BLK, D], F32, tag="pb")
                nc.tensor.transpose(ps_o[:], xT[:, b * BLK:(b + 1) * BLK],
                                    ident_s[:])
                ot = gp.tile([BLK, D], F32, tag="ot")
                nc.scalar.copy(ot[:], ps_o[:])
                nc.sync.dma_start(out[b * BLK:(b + 1) * BLK, :], ot[:])

    nc.compile()
    return nc


FULL_CFG = dict(N=20000, E=320000, CORES=8, SHARD=2500, BLK=125, BLOCKS=20,
                C=None, L=3, D=128, H=4, NQ=4)


def kernel_run(inputs, trace=False):
    cfg = dict(FULL_CFG)
    in_maps, C = host_prep(inputs, cfg)
    nc = build_program(cfg)
    res = run_bass_kernel_spmd(nc, in_maps, list(range(cfg["CORES"])),
                               trace=trace)
    out = np.concatenate([r["out"] for r in res.results], axis=0)
    return out, res


def kernel(**inputs):
    out, _ = kernel_run(inputs)
    return out.astype(np.float32)


# revision 25
# speedup vs baseline: 1.5064x; 1.5064x over previous
"""Trainium2 Bass kernel for a 3-layer GAT block (DeepGATBlockV2).

Strategy (8-core SPMD, nodes partitioned by dst range):
  - ALL per-core constants + inputs are packed into ONE int16 blob input.
  - Per layer, each core builds packed table rows for its 2500-node shard:
    256 B/row = [x as fp8e4 (128 B) | a_s bf16 (8 B) | a_d bf16 (8 B) |
    pad (112 B)].  AllGather -> full [N, 256 B] table in DRAM (5.12 MB).
  - Edges (incl. self loops) are sorted by dst on the host, bucketed into
    per-core dst-blocks of 125 nodes, padded to a uniform C chunks of 128
    edge slots.  Per block ONE merged dma_gather fetches 2*slots rows
    (src rows then dst rows, 256 B each).
  - q = a_s[src]+a_d[dst] (DVE 2x bf16); w = exp(leaky_relu(q)) on ACT,
    written twice (wex2 pairs) so the one-hot builds hit the DVE 2x mode:
    s0[e, (n/2, 2)] and sh[e, (h, n/2, 2)] use pair-padded width NB=126
    with all-packed innermost dims.  PE accumulates
    s_hT[f, (h,nb)] += Xg(fp8).T @ sh(bf16) and den[(h,nb)] += wex @ s0.
  - Block post: y = s_hT * bcast(1/den) (0.25 head-mean folded into
    W_gat), attT = sum_h W_h.T @ y_h + bias; residual + RMSNorm + FFN
    (bf16 matmuls) + RMSNorm, all feature-major.  Phase A of the NEXT
    layer is emitted right after each block's post so table rows stream
    out while later blocks still aggregate.
  - RMSNorm rsqrt = exp(-0.5*ln(ms+eps)); one activation table set.
"""

import functools

import numpy as np

import concourse.bass as bass
import concourse.bacc as bacc
import concourse.hw_specs as hw_specs
import concourse.tile as tile
from concourse import mybir
from concourse.bass_utils import run_bass_kernel_spmd

F32 = mybir.dt.float32
BF16 = mybir.dt.bfloat16
I16 = mybir.dt.int16
I8 = mybir.dt.int8
F8 = mybir.dt.float8e4
AOT = mybir.AluOpType
ACT = mybir.ActivationFunctionType

import os

EPS = 1.1920929e-07
NEG_SLOPE = 0.2
ABLATE = set(os.environ.get("ABL", "").split(",")) - {""}  # ablation flags

# ---- activation-table forcing ----------------------------------------
_COMBINED_SET = "natural_log_exp_and_others"
_orig_get_tables = hw_specs.get_activation_tables


@functools.cache
def _forced_tables(arch):
    t = {k: set(v) for k, v in _orig_get_tables(arch).items()}
    used = {ACT.Exp, ACT.Ln, ACT.Prelu, ACT.Relu, ACT.Copy, ACT.Identity,
            ACT.MemsetZero}
    if _COMBINED_SET in t and used <= t[_COMBINED_SET]:
        for name, funcs in t.items():
            if name != _COMBINED_SET:
                funcs -= used
    return t


hw_specs.get_activation_tables = _forced_tables
bacc.get_activation_tables = _forced_tables


def _np_bf16():
    import ml_dtypes
    return ml_dtypes.bfloat16


class _Blob:
    """Packs 2-D arrays into one flat int16 buffer, 256B-aligned rows."""

    def __init__(self):
        self.parts = []
        self.off = 0  # int16 elems
        self.secs = {}

    def add(self, name, arr):
        a = np.ascontiguousarray(arr)
        b = a.view(np.int16)
        rows, cols = b.shape
        pad = (-self.off) % 128
        if pad:
            self.parts.append(np.zeros(pad, np.int16))
            self.off += pad
        self.secs[name] = (self.off, rows, cols)
        self.parts.append(b.reshape(-1))
        self.off += rows * cols

    def finish(self):
        pad = (-self.off) % 128
        if pad:
            self.parts.append(np.zeros(pad, np.int16))
            self.off += pad
        return np.concatenate(self.parts)


def host_prep(inputs, cfg):
    """Returns (in_maps, C) -- per-core single-blob inputs."""
    N, E, CORES = cfg["N"], cfg["E"], cfg["CORES"]
    SHARD, BLK, BLOCKS = cfg["SHARD"], cfg["BLK"], cfg["BLOCKS"]
    L, D, H = cfg["L"], cfg["D"], cfg["H"]
    NB = BLK + 1  # pair-padded one-hot width (126)
    bf16 = _np_bf16()

    x = np.ascontiguousarray(np.asarray(inputs["x"], np.float32))
    ei = np.asarray(inputs["edge_index"], np.int64)
    src = ei[0]
    dst = ei[1]
    loops = np.arange(N, dtype=np.int64)
    src = np.concatenate([src, loops])
    dst = np.concatenate([dst, loops])
    order = np.argsort(dst, kind="stable")
    src, dst = src[order], dst[order]

    nblk_total = N // BLK
    blk_of = dst // BLK
    counts = np.bincount(blk_of, minlength=nblk_total)
    C = int(np.ceil(counts.max() / 128))
    cfg["C"] = C
    slots = C * 128

    srcs = np.zeros((CORES, BLOCKS, slots), np.int64)
    dloc = np.full((CORES, BLOCKS, slots), -1.0, np.float32)
    dsts = np.zeros((CORES, BLOCKS, slots), np.int64)
    starts = np.concatenate([[0], np.cumsum(counts)])
    for b in range(nblk_total):
        core, blk = b // BLOCKS, b % BLOCKS
        s, e = int(starts[b]), int(starts[b + 1])
        n = e - s
        srcs[core, blk, :n] = src[s:e]
        dsts[core, blk, :n] = dst[s:e]
        dsts[core, blk, n:] = b * BLK  # valid row for pad reads
        dloc[core, blk, :n] = (dst[s:e] - b * BLK).astype(np.float32)

    # merged per-block index list: [srcs | dsts], wrapped for dma_gather
    both = np.concatenate([srcs, dsts], axis=2)  # [CORES, BLOCKS, 2*slots]

    def wrap_idx(a):
        # a: [BLOCKS, ns] int -> int16 [128, BLOCKS * ns//16]
        ns = a.shape[1]
        a16 = a.reshape(BLOCKS, ns // 16, 16).transpose(0, 2, 1)
        a16 = a16.reshape(1, BLOCKS * 16, ns // 16)
        cols = np.concatenate(
            [a16[0, b * 16:(b + 1) * 16, :] for b in range(BLOCKS)],
            axis=1)  # [16, BLOCKS*ns//16]
        assert a.max() < 2 ** 15
        return np.tile(cols.astype(np.int16), (8, 1))

    # dloc layout [128, BLOCKS*C]: [p, b*C + ch] = slot ch*128+p of block b
    dloc_t = dloc.reshape(CORES, BLOCKS, C, 128).transpose(0, 3, 1, 2) \
                 .reshape(CORES, 128, BLOCKS * C)
    dloc2 = np.repeat(dloc_t, 2, axis=2)  # [CORES, 128, BLOCKS*C*2]
    # flat slot-order dloc per block: [1, BLOCKS*slots]
    dlocf = dloc.reshape(CORES, 1, BLOCKS * slots)

    Wg = np.asarray(inputs["W_gat"], np.float32)     # [L, D, H*D]
    a_s = np.asarray(inputs["att_src"], np.float32)  # [L, H, D]
    a_d = np.asarray(inputs["att_dst"], np.float32)
    wasd = np.zeros((D, L * 2 * H), np.float32)
    for l in range(L):
        for h in range(H):
            Wh = Wg[l][:, h * D:(h + 1) * D]
            wasd[:, l * 2 * H + h] = Wh @ a_s[l, h]
            wasd[:, l * 2 * H + H + h] = Wh @ a_d[l, h]

    def col3(name):  # [L, D] -> [D, L]
        return np.ascontiguousarray(np.asarray(inputs[name], np.float32).T)

    blob = _Blob()
    # 0.25 head-mean folded into W_gat; [D, L*H*D] d-major
    blob.add("wgat", (0.25 * Wg).transpose(1, 0, 2).reshape(D, L * H * D)
             .astype(bf16))
    blob.add("w1", np.asarray(inputs["W1"], np.float32)
             .transpose(1, 0, 2).reshape(D, L * D).astype(bf16))
    blob.add("w2", np.asarray(inputs["W2"], np.float32)
             .transpose(1, 0, 2).reshape(D, L * D).astype(bf16))
    blob.add("wasd", wasd)
    blob.add("bg", col3("bias_gat"))
    blob.add("b1", col3("b1"))
    blob.add("b2", col3("b2"))
    blob.add("n1", np.asarray(inputs["norm1_w"], np.float32).reshape(1, -1))
    blob.add("n2", np.asarray(inputs["norm2_w"], np.float32).reshape(1, -1))
    blob.add("iota", np.tile(np.arange(NB, dtype=np.float32), (128, 1))
             .astype(bf16))
    blob.add("iotap", np.tile(np.arange(BLK, dtype=np.float32)[:, None],
                              (1, 2)).astype(bf16))
    blob.add("ident", np.eye(128, dtype=np.float32))
    blob.add("onesf", np.ones((128, 2), np.float32))  # col 0 used
    # head-selector for denominator broadcast: hsel[k, h*128+m] = (k==h)
    blob.add("hsel", np.eye(H, dtype=np.float32).repeat(128, axis=1)
             .astype(bf16))
    common_len = blob.off
    common_parts = list(blob.parts)
    common_secs = dict(blob.secs)

    in_maps = []
    blob_len = None
    for c in range(CORES):
        bl = _Blob()
        bl.parts = list(common_parts)
        bl.off = common_len
        bl.secs = dict(common_secs)
        bl.add("idx", wrap_idx(both[c]))
        bl.add("dloc2", dloc2[c].astype(bf16))
        bl.add("dlocf", dlocf[c].astype(bf16))
        bl.add("xin", x[c * SHARD:(c + 1) * SHARD])
        buf = bl.finish()
        blob_len = len(buf)
        cfg["SECS"] = bl.secs
        in_maps.append({"blob": buf})
    cfg["BLOB_LEN"] = blob_len
    return in_maps, C


def build_program(cfg, debug=False):
    N, CORES = cfg["N"], cfg["CORES"]
    SHARD, BLK, BLOCKS, C = cfg["SHARD"], cfg["BLK"], cfg["BLOCKS"], cfg["C"]
    L, D, H = cfg["L"], cfg["D"], cfg["H"]
    SECS = cfg["SECS"]
    NB = BLK + 1      # pair-padded block width (126)
    TB = 256          # table row bytes
    slots = C * 128
    NQ = cfg.get("NQ", 4)
    RP = cfg.get("REPS", 1)

    nc = bacc.Bacc("TRN2", target_bir_lowering=False, debug=debug,
                   num_devices=CORES, num_swdge_queues=NQ,
                   dynamic_dma_scratch_size=36864)

    blob = nc.dram_tensor("blob", [cfg["BLOB_LEN"]], I16,
                          kind="ExternalInput").ap()
    out = nc.dram_tensor("out", [SHARD, D], F32, kind="ExternalOutput").ap()

    def sec(name, dt=F32):
        off, rows, cols = SECS[name]
        v = blob[off:off + rows * cols].rearrange("(r c) -> r c", c=cols)
        return v if dt == I16 else v.bitcast(dt)

    with tile.TileContext(nc) as tc:
        with tc.tile_pool(name="persist", bufs=1) as pp, \
             tc.tile_pool(name="dram", bufs=1, space="DRAM") as dp, \
             tc.tile_pool(name="gath", bufs=4) as gp, \
             tc.tile_pool(name="sc", bufs=4) as scp, \
             tc.tile_pool(name="chunk", bufs=2) as cp, \
             tc.tile_pool(name="rep", bufs=2) as rp, \
             tc.tile_pool(name="post", bufs=2) as pop, \
             tc.tile_pool(name="psA", bufs=1, space="PSUM") as psA, \
             tc.tile_pool(name="psB", bufs=2, space="PSUM") as psB:

            # ---- persistent SBUF ----
            idx_s = pp.tile([128, BLOCKS * 2 * slots // 16], I16)
            dloc2_s = pp.tile([128, BLOCKS * C * 2], BF16)
            wasd_s = pp.tile([128, L * 2 * H], F32)
            wgat_s = pp.tile([128, L * H * D], BF16)
            w1_s = pp.tile([128, L * D], BF16)
            w2_s = pp.tile([128, L * D], BF16)
            bg_s = pp.tile([128, L], F32)
            b1_s = pp.tile([128, L], F32)
            b2_s = pp.tile([128, L], F32)
            n1_s = pp.tile([1, L * D], F32)
            n2_s = pp.tile([1, L * D], F32)
            iota_s = pp.tile([128, NB], BF16)
            iotap_s = pp.tile([BLK, 2], BF16)
            asd_s = pp.tile([BLK, 2 * BLOCKS * H], BF16)
            ident_s = pp.tile([128, 128], F32)
            onesf_s = pp.tile([128, 2], F32)
            hsel_s = pp.tile([4, 4 * 128], BF16)
            xT = pp.tile([128, SHARD], F32)
            eps_s = pp.tile([1, 1], F32)

            dma = nc.sync.dma_start
            dma(idx_s[:], sec("idx", I16))
            dma(dloc2_s[:], sec("dloc2", BF16))
            dma(wasd_s[:], sec("wasd"))
            dma(wgat_s[:], sec("wgat", BF16))
            dma(w1_s[:], sec("w1", BF16))
            dma(w2_s[:], sec("w2", BF16))
            dma(bg_s[:], sec("bg"))
            dma(b1_s[:], sec("b1"))
            dma(b2_s[:], sec("b2"))
            dma(n1_s[:], sec("n1"))
            dma(n2_s[:], sec("n2"))
            dma(iota_s[:], sec("iota", BF16))
            dma(iotap_s[:], sec("iotap", BF16))
            dma(ident_s[:], sec("ident"))
            dma(onesf_s[:], sec("onesf"))
            dma(hsel_s[:], sec("hsel", BF16))
            nc.vector.memset(eps_s[:], EPS)
            xin = sec("xin", F32)

            # ---- DRAM tables for gather + collective ----
            tshard = dp.tile([SHARD, TB], I8)
            if CORES > 1:
                tfulls = [dp.tile([N, TB], I8, addr_space="Shared",
                                  tag=f"tfull{i}", name=f"tfull{i}")
                          for i in range(L * RP)]
            else:
                tfulls = [tshard] * (L * RP)

            # ---- init: transpose input shard to feature-major xT ----
            for b in range(BLOCKS):
                xr = gp.tile([BLK, D], F32, tag="xr")
                nc.sync.dma_start(xr[:], xin[b * BLK:(b + 1) * BLK, :])
                ps_t = psB.tile([D, BLK], F32, tag="pb")
                nc.tensor.transpose(ps_t[:], xr[:], ident_s[:BLK, :BLK])
                nc.scalar.copy(xT[:, b * BLK:(b + 1) * BLK], ps_t[:])

            def rmsnorm(z, nw_row, tag, zout=None):
                """z: SBUF [D, BLK] f32 -> z * rsqrt(mean(z^2)+eps) * w."""
                zsq = pop.tile([D, BLK], F32, tag=f"zsq{tag}")
                nc.vector.tensor_mul(zsq[:], z[:], z[:])
                ps_ss = psB.tile([1, BLK], F32, tag="pb_ss", bufs=1)
                nc.tensor.matmul(ps_ss[:], onesf_s[:, 0:1], zsq[:],
                                 start=True, stop=True)
                lnm = pop.tile([1, BLK], F32, tag=f"lnm{tag}")
                nc.scalar.activation(lnm[:], ps_ss[:], ACT.Ln,
                                     scale=1.0 / D, bias=eps_s[:])
                rin = pop.tile([1, BLK], F32, tag=f"rin{tag}")
                nc.scalar.activation(rin[:], lnm[:], ACT.Exp, scale=-0.5)
                ps_rb = psB.tile([D, BLK], F32, tag="pb")
                nc.tensor.matmul(ps_rb[:], nw_row, rin[:],
                                 start=True, stop=True)
                zn = zout if zout is not None else pop.tile(
                    [D, BLK], F32, tag=f"zn{tag}")
                nc.vector.tensor_mul(zn if zout is not None else zn[:],
                                     z[:], ps_rb[:])
                return zn

            def phase_a(l, b):
                """Write packed table rows for block b, layer l."""
                xb = xT[:, b * BLK:(b + 1) * BLK]
                ps_a = psB.tile([BLK, 2 * H], F32, tag="pb")
                nc.tensor.matmul(ps_a[:], xb,
                                 wasd_s[:, l * 2 * H:(l + 1) * 2 * H],
                                 start=True, stop=True)
                ps_x = psB.tile([BLK, D], F32, tag="pb")
                nc.tensor.transpose(ps_x[:], xb, ident_s[:])
                tt = gp.tile([BLK, TB], I8, tag="tt")
                nc.scalar.copy(tt[:, 0:128].bitcast(F8), ps_x[:])
                nc.scalar.copy(tt[:, 128:144].bitcast(BF16), ps_a[:])
                nc.scalar.copy(
                    asd_s[:, ((l % 2) * BLOCKS + b) * H:
                          ((l % 2) * BLOCKS + b + 1) * H],
                    ps_a[:, H:2 * H])
                nc.vector.memset(tt[:, 144:TB], 0)
                nc.sync.dma_start(tshard[b * BLK:(b + 1) * BLK, :], tt[:])

            # ---- phase C1: issue the gathers for one block ----
            def gather_phase(i, b):
                tf = tfulls[i]
                gab = gp.tile([128, C * TB], I8, tag="gab")
                gv = gab[:].rearrange("p (c e) -> p c e", e=TB)
                ic0 = b * (2 * slots // 16)
                icm = ic0 + slots // 16
                if "ga" not in ABLATE:
                    nc.gpsimd.dma_gather(
                        gv[:, 0:C, :], tf[:], idx_s[:, ic0:icm],
                        num_idxs=slots, num_idxs_reg=slots,
                        elem_size=TB, queue_num=b % NQ,
                        single_packet="spkt" in ABLATE)
                else:
                    nc.vector.memset(gv[:, 0:1, 0:4], 0)
                # per-edge a_d via one-hot-transpose matmuls (no dst gather)
                dlf = rp.tile([1, slots], BF16, tag="dlf")
                nc.sync.dma_start(
                    dlf[:], sec("dlocf", BF16)[0:1, b * slots:(b + 1) * slots])
                rep = rp.tile([BLK, slots], BF16, tag="rep")
                nc.gpsimd.partition_broadcast(rep[:], dlf[:])
                s0T = rp.tile([BLK, slots], BF16, tag="s0T")
                nc.vector.tensor_tensor(
                    s0T[:].rearrange("p (j two) -> p j two", two=2),
                    rep[:].rearrange("p (j two) -> p j two", two=2),
                    iotap_s[:].unsqueeze(1)
                        .broadcast_to([BLK, slots // 2, 2]),
                    op=AOT.is_equal)
                ps_ad = psB.tile([128, C * H], F32, tag="pb_ad", bufs=1)
                av = asd_s[:, ((i % L) % 2) * BLOCKS * H:]
                for ch in range(C):
                    nc.tensor.matmul(
                        ps_ad[:, ch * H:(ch + 1) * H],
                        s0T[:, ch * 128:(ch + 1) * 128],
                        av[:, b * H:(b + 1) * H],
                        start=True, stop=True)
                qd = scp.tile([128, C * H], BF16, tag="qd")
                nc.scalar.copy(qd[:], ps_ad[:])
                return gv, qd
                return gv

            # ---- phase C2a: per-edge attention scores (DVE+ACT only) ----
            def score_phase(b, gv, qd):
                q = scp.tile([128, C * H], BF16, tag="q")
                lr = scp.tile([128, C * H], BF16, tag="lr")
                wex2 = scp.tile([128, C * H * 2], BF16, tag="wex2")
                nc.vector.tensor_add(
                    q[:].rearrange("p (c h) -> p c h", h=H),
                    gv[:, 0:C, 128:136].bitcast(BF16),
                    qd[:].rearrange("p (c h) -> p c h", h=H))
                nc.scalar.activation(lr[:], q[:], ACT.Prelu,
                                     alpha=NEG_SLOPE)
                w2v = wex2[:].rearrange("p (c h two) -> p c h two", h=H,
                                        two=2)
                nc.scalar.activation(
                    w2v[:, :, :, 0].rearrange("p c h -> p (c h)"),
                    lr[:], ACT.Exp)
                nc.scalar.activation(
                    w2v[:, :, :, 1].rearrange("p c h -> p (c h)"),
                    lr[:], ACT.Exp)
                return w2v

            # ---- phase C2b: one-hots + PE aggregation ----
            def math_phase(i, b, gv, w2v):
                s0 = cp.tile([128, C * NB], BF16, tag="s0")
                sh = cp.tile([128, C * H * NB], BF16, tag="sh")
                nc.vector.tensor_tensor(
                    s0[:].rearrange("p (c n2 two) -> p c n2 two", two=2,
                                    n2=NB // 2),
                    dloc2_s[:, b * C * 2:(b + 1) * C * 2]
                        .rearrange("p (c two) -> p c two", two=2)
                        .unsqueeze(2).broadcast_to([128, C, NB // 2, 2]),
                    iota_s[:].rearrange("p (n2 two) -> p n2 two", two=2)
                        .unsqueeze(1).broadcast_to([128, C, NB // 2, 2]),
                    op=AOT.is_equal)
                C2 = C // 2
                for c0, c1 in ((0, C2), (C2, C)):
                    cw = c1 - c0
                    nc.vector.tensor_tensor(
                        sh[:, c0 * H * NB:c1 * H * NB]
                            .rearrange("p (c h n2 two) -> p c h n2 two",
                                       h=H, two=2, n2=NB // 2),
                        s0[:, c0 * NB:c1 * NB]
                            .rearrange("p (c n2 two) -> p c n2 two", two=2,
                                       n2=NB // 2)
                            .unsqueeze(2)
                            .broadcast_to([128, cw, H, NB // 2, 2]),
                        w2v[:, c0:c1]
                            .unsqueeze(3)
                            .broadcast_to([128, cw, H, NB // 2, 2]),
                        op=AOT.mult)

                ps_all = psA.tile([D, H * NB], F32, tag="ps_all",
                                  name=f"ps_all_{i}_{b}", bufs=2)
                ps_den = psA.tile([H, NB], F32, tag="ps_den",
                                  name=f"ps_den_{i}_{b}", bufs=2)
                wex0 = w2v[:, :, :, 0]
                # den first: it only needs s0+wex, so PE can run it while
                # the big sh build is still in flight on DVE
                for ch in range(C):
                    nc.tensor.matmul(
                        ps_den[:], wex0[:, ch, :],
                        s0[:, ch * NB:(ch + 1) * NB],
                        start=(ch == 0), stop=(ch == C - 1))
                for ch in range(C):
                    nc.tensor.matmul(
                        ps_all[:], gv[:, ch, 0:128].bitcast(F8),
                        sh[:, ch * H * NB:(ch + 1) * H * NB],
                        start=(ch == 0), stop=(ch == C - 1))
                # 1/den broadcast to 128 partitions (runs during agg on PE)
                rden = pop.tile([H, BLK], BF16, tag="rden")
                with nc.allow_low_precision(reason="1/den fine in bf16"):
                    nc.vector.reciprocal(rden[:], ps_den[:, 0:BLK])
                ps_rb = psB.tile([128, H * BLK], F32, tag="pb")
                for h in range(H):
                    nc.tensor.matmul(ps_rb[:, h * BLK:(h + 1) * BLK],
                                     hsel_s[:, h * 128:(h + 1) * 128],
                                     rden[:], start=True, stop=True)
                rb = pop.tile([128, H * BLK], F32, tag="rb", bufs=4)
                nc.scalar.copy(rb[:], ps_rb[:])
                return ps_all, rb

            # ---- phase D: softmax-normalize + FFN for one block ----
            def post_phase(l, b, ps_all, rb):
                xb = xT[:, b * BLK:(b + 1) * BLK]
                yh = pop.tile([128, H * BLK], BF16, tag="yh")
                nc.vector.tensor_mul(
                    yh[:].rearrange("p (h n) -> p h n", n=BLK),
                    ps_all[:].rearrange("p (h n) -> p h n", n=NB)
                        [:, :, 0:BLK],
                    rb[:].rearrange("p (h n) -> p h n", n=BLK))
                ps_att = psB.tile([D, BLK], F32, tag="pb")
                for h in range(H):
                    nc.tensor.matmul(
                        ps_att[:],
                        wgat_s[:, (l * H + h) * D:(l * H + h + 1) * D],
                        yh[:, h * BLK:(h + 1) * BLK],
                        start=(h == 0), stop=(h == H - 1))

                z = pop.tile([D, BLK], F32, tag="z")
                nc.vector.scalar_tensor_tensor(
                    z[:], ps_att[:], bg_s[:, l:l + 1], xb,
                    op0=AOT.add, op1=AOT.add)
                zn1 = rmsnorm(z, n1_s[0:1, l * D:(l + 1) * D], "a")

                zn1h = pop.tile([D, BLK], BF16, tag="zn1h")
                nc.scalar.copy(zn1h[:], zn1[:])
                ps_f1 = psB.tile([D, BLK], F32, tag="pb")
                nc.tensor.matmul(ps_f1[:], w1_s[:, l * D:(l + 1) * D],
                                 zn1h[:], start=True, stop=True)
                f1 = pop.tile([D, BLK], BF16, tag="f1")
                nc.scalar.activation(f1[:], ps_f1[:], ACT.Relu,
                                     bias=b1_s[:, l:l + 1])
                ps_f2 = psB.tile([D, BLK], F32, tag="pb")
                nc.tensor.matmul(ps_f2[:], w2_s[:, l * D:(l + 1) * D],
                                 f1[:], start=True, stop=True)
                z3 = pop.tile([D, BLK], F32, tag="z3")
                nc.vector.scalar_tensor_tensor(
                    z3[:], ps_f2[:], b2_s[:, l:l + 1], zn1[:],
                    op0=AOT.add, op1=AOT.add)
                rmsnorm(z3, n2_s[0:1, l * D:(l + 1) * D], "b", zout=xb)

            # ---- main sequence ----
            for b in range(BLOCKS):
                phase_a(0, b)
            NLAY = L * RP
            AHEAD = 2  # gathers issued this many blocks early
            for i in range(NLAY):
                l = i % L
                if CORES > 1 and "coll" not in ABLATE:
                    nc.gpsimd.collective_compute(
                        "AllGather", AOT.bypass,
                        replica_groups=[list(range(CORES))],
                        ins=[tshard.opt()], outs=[tfulls[i].opt()])
                gvs = {b: gather_phase(i, b) for b in range(AHEAD)}
                pend = []  # lag-2 post queue
                for b in range(BLOCKS):
                    if b + AHEAD < BLOCKS:
                        gvs[b + AHEAD] = gather_phase(i, b + AHEAD)
                    gv, qd = gvs.pop(b)
                    w2v = score_phase(b, gv, qd)
                    if len(pend) >= 2:
                        pb_, psall_, rb_ = pend.pop(0)
                        post_phase(l, pb_, psall_, rb_)
                        if i + 1 < NLAY:
                            phase_a((i + 1) % L, pb_)
                    pend.append((b, *math_phase(i, b, gv, w2v)))
                for pb_, psall_, rb_ in pend:
                    post_phase(l, pb_, psall_, rb_)
                    if i + 1 < NLAY:
                        phase_a((i + 1) % L, pb_)

            # ---- output: transpose back to node-major ----
            for b in range(BLOCKS):
                ps_o = psB.tile([BLK, D], F32, tag="pb")
                nc.tensor.transpose(ps_o[:], xT[:, b * BLK:(b + 1) * BLK],
                                    ident_s[:])
                ot = gp.tile([BLK, D], F32, tag="ot")
                nc.scalar.copy(ot[:], ps_o[:])
                nc.sync.dma_start(out[b * BLK:(b + 1) * BLK, :], ot[:])

    nc.compile()
    return nc


FULL_CFG = dict(N=20000, E=320000, CORES=8, SHARD=2500, BLK=125, BLOCKS=20,
                C=None, L=3, D=128, H=4, NQ=4)


def kernel_run(inputs, trace=False):
    cfg = dict(FULL_CFG)
    in_maps, C = host_prep(inputs, cfg)
    nc = build_program(cfg)
    res = run_bass_kernel_spmd(nc, in_maps, list(range(cfg["CORES"])),
                               trace=trace)
    out = np.concatenate([r["out"] for r in res.results], axis=0)
    return out, res


def kernel(**inputs):
    out, _ = kernel_run(inputs)
    return out.astype(np.float32)
